# revision 55
# baseline (speedup 1.0000x reference)
"""Chamfer distance kernel for Trainium2 (8 NeuronCores).

Problem: x, y ~ (B=4, N=8192, 3) fp32. loss = mean_b[ mean_n min_m dist + mean_m min_n dist ].

Strategy:
  - d2[n,m] = ||x||^2 + ||y||^2 - 2 x.y as ONE augmented matmul: the fp32
    values are split hi/lo into fp16 (exact products in PSUM fp32), giving
    K=13 fp16 contraction rows -> 4x faster PE than fp32, ~1e-7 accuracy.
  - K=13 <= 32, so 4 independent matmuls are packed into the PE array via
    tile_position row-groups (base partitions 0/32/64/96) -> ~4x concurrency.
  - min over the free axis; sqrt/means on host (monotone => sqrt after min).
  - Two consumer lanes, balanced so DVE and ACT both stay busy. Every block
    (n-tile x full 8192 cols) is handled identically to avoid convoying:
      4 chunks x 1536: ScalarE Relu-cast-copies PSUM->SBUF fp16; DVE then
                       folds the 6144 cols with tensor_tensor min (fp16 2x)
      2 chunks x 1024: DVE reduce_min straight from PSUM
    Per-block partials land in a tails buffer; one batched reduce per 8
    blocks produces the final row-mins. Folds are emitted one block late so
    DVE frees PSUM promptly (keeps the ACT chunk stream gap-free).
    (tensor_tensor_reduce would be 1 instr/block but crashes this runtime)
  - 8 cores = (batch b, half h): each core does both directions for its half
    of the rows against the full opposite set => no cross-core reduction.
"""

import sys

sys.path.insert(0, "/opt/trn_rl_repo")

import numpy as np

import concourse.bacc as bacc
import concourse.mybir as mybir
from concourse import tile
from concourse.bass_utils import run_bass_kernel_spmd

N_CORES = 8
B, N, M, D = 4, 8192, 8192, 3
HALF = N // 2  # rows per core per direction
NT = HALF // 128  # 32 n-tiles per pass
import os as _os

K = 13  # augmented fp16 hi/lo contraction rows
# per-block chunk plan: ACT copies ACT_N x ACT_CHUNK cols PSUM->SBUF fp16,
# DVE reduces DIR_N x DIR_CHUNK cols directly from PSUM.
# PSUM banks: ACT_N needs 2 bufs x ACT_CHUNK/512 banks, DIR 2 x DIR_CHUNK/512.
ACT_CHUNK = int(_os.environ.get("CHAMFER_ACT_CHUNK", "1536"))
ACT_N = int(_os.environ.get("CHAMFER_ACT_N", "4"))
DIR_CHUNK = int(_os.environ.get("CHAMFER_DIR_CHUNK", "1024"))
DIR_N = int(_os.environ.get("CHAMFER_DIR_N", "2"))
assert ACT_CHUNK * ACT_N + DIR_CHUNK * DIR_N == M
STAGE_BUFS = int(_os.environ.get("CHAMFER_STAGE_BUFS", "2"))
TGROUP = int(_os.environ.get("CHAMFER_TGROUP", "8"))  # blocks per batched tail-reduce
F32 = mybir.dt.float32
F16 = mybir.dt.float16

_NC_CACHE = {}


def build_bass():
    nc = bacc.Bacc(
        "TRN2", target_bir_lowering=False, debug=False, num_devices=N_CORES
    )
    la = nc.dram_tensor("la", [K, HALF], F16, kind="ExternalInput")
    ra0 = nc.dram_tensor("ra0", [K, M // 2], F16, kind="ExternalInput")
    ra1 = nc.dram_tensor("ra1", [K, M // 2], F16, kind="ExternalInput")
    lb = nc.dram_tensor("lb", [K, HALF], F16, kind="ExternalInput")
    rb0 = nc.dram_tensor("rb0", [K, M // 2], F16, kind="ExternalInput")
    rb1 = nc.dram_tensor("rb1", [K, M // 2], F16, kind="ExternalInput")
    out = nc.dram_tensor("out", [128, 2 * NT], F32, kind="ExternalOutput")

    with tile.TileContext(nc) as tc:
        with (
            tc.tile_pool(name="inp", bufs=1) as inp,
            tc.tile_pool(name="psa", bufs=2, space="PSUM") as psa,
            tc.tile_pool(
                name="psd",
                bufs=int(_os.environ.get("CHAMFER_DIR_BUFS", "1")),
                space="PSUM",
            ) as psd,
            tc.tile_pool(name="stg", bufs=STAGE_BUFS) as stg,
            tc.tile_pool(name="foldp", bufs=int(_os.environ.get("CHAMFER_FOLD_BUFS", "2"))) as foldp,
            tc.tile_pool(name="tailp", bufs=int(_os.environ.get("CHAMFER_TAIL_BUFS", "2"))) as tailp,
            tc.tile_pool(name="res", bufs=1) as resp,
        ):
            # warm the ACT table (Relu set) while input DMAs run
            warm = inp.tile([128, 1], F32, tag="warm")
            nc.vector.memset(warm[:], 0.0)
            nc.scalar.activation(warm[:], warm[:], mybir.ActivationFunctionType.Relu)

            # inputs replicated at base partitions 0/32/64/96 for row-group
            # packed matmuls (4 concurrent MMs in the PE array)
            ls_a = inp.tile([128, HALF], F16, tag="la")
            rs_a = inp.tile([128, M], F16, tag="ra")
            ls_b = inp.tile([128, HALF], F16, tag="lb")
            rs_b = inp.tile([128, M], F16, tag="rb")
            for g in range(4):  # pass-A inputs first: block 0 starts sooner
                p0 = 32 * g
                nc.sync.dma_start(ls_a[p0 : p0 + K, :], la[:])
                nc.sync.dma_start(rs_a[p0 : p0 + K, : M // 2], ra0[:])
            for g in range(4):
                p0 = 32 * g
                nc.sync.dma_start(rs_a[p0 : p0 + K, M // 2 :], ra1[:])
            for g in range(4):
                p0 = 32 * g
                nc.sync.dma_start(ls_b[p0 : p0 + K, :], lb[:])
                nc.sync.dma_start(rs_b[p0 : p0 + K, : M // 2], rb0[:])
                nc.sync.dma_start(rs_b[p0 : p0 + K, M // 2 :], rb1[:])

            out_s = resp.tile([128, 2 * NT], F32)

            ndirect = DIR_N
            sw = ACT_N * ACT_CHUNK
            tailw = sw >> 4  # 4 fp16 TT-min fold levels
            blocks = [(p, t) for p in range(2) for t in range(NT)]
            ngroups = len(blocks) // TGROUP

            def emit_fold(stile, tails, kk):
                fold = foldp.tile([128, sw // 2], F16, tag="fold")
                cur, cw = stile, sw
                for _ in range(4):
                    half = cw // 2
                    dst = (
                        tails[:, kk, ndirect : ndirect + tailw]
                        if half == tailw
                        else fold[:, :half]
                    )
                    nc.vector.tensor_tensor(
                        dst, cur[:, :half], cur[:, half:cw], op=mybir.AluOpType.min
                    )
                    cur, cw = fold, half

            def emit_tailred(gi, tails, lo=0, hi=TGROUP):
                nc.vector.tensor_reduce(
                    out_s[:, gi * TGROUP + lo : gi * TGROUP + hi],
                    tails[:, lo:hi, :],
                    axis=mybir.AxisListType.X,
                    op=mybir.AluOpType.min,
                )

            # software pipeline: folds run one block late, tail-reduces one
            # block into the next group, so DVE frees PSUM chunks promptly
            pending_fold = None
            group_tails = {}
            for idx, (p, t) in enumerate(blocks):
                gi, kk = divmod(idx, TGROUP)
                if kk == 0:
                    group_tails[gi] = tailp.tile(
                        [128, TGROUP, ndirect + tailw], F16, tag="tails", name="tails"
                    )
                tails = group_tails[gi]
                lt, rt = (ls_a, rs_a) if p == 0 else (ls_b, rs_b)
                stile = stg.tile([128, sw], F16, tag="stage")
                gseq = 0
                acts = [("act", j, ACT_CHUNK, j * ACT_CHUNK) for j in range(ACT_N)]
                dves = [
                    ("dve", j, DIR_CHUNK, sw + j * DIR_CHUNK) for j in range(DIR_N)
                ]
                order = _os.environ.get("CHAMFER_ORDER", "head")
                if order == "tail":
                    chunks = acts + dves
                elif order == "head":
                    chunks = dves + acts
                else:  # interleave: a a d a a d
                    chunks = acts[:2] + dves[:1] + acts[2:] + dves[1:]
                for kind, j, csz, c0 in chunks:
                    if kind == "act":
                        ps = psa.tile([128, ACT_CHUNK], F32, name="psa_t", tag="psa_t")
                    else:
                        ps = psd.tile([128, DIR_CHUNK], F32, name="psd_t", tag="psd_t")
                    for s in range(csz // 512):
                        g = gseq % 4  # rotate PE row-groups
                        gseq += 1
                        p0 = 32 * g
                        cs = c0 + s * 512
                        nc.tensor.matmul(
                            ps[:, s * 512 : (s + 1) * 512],
                            lt[p0 : p0 + K, t * 128 : (t + 1) * 128],
                            rt[p0 : p0 + K, cs : cs + 512],
                            tile_position=(p0, 0),
                        )
                    if kind == "act":
                        nc.scalar.activation(
                            stile[:, j * ACT_CHUNK : (j + 1) * ACT_CHUNK],
                            ps[:],
                            mybir.ActivationFunctionType.Relu,
                        )
                    else:
                        nc.vector.tensor_reduce(
                            tails[:, kk, j : j + 1],
                            ps[:],
                            axis=mybir.AxisListType.X,
                            op=mybir.AluOpType.min,
                        )
                if pending_fold is not None:
                    emit_fold(*pending_fold)
                pending_fold = (stile, tails, kk)
                # split each group's tail reduce into two halves emitted on
                # different blocks: smaller DVE bursts -> fewer ACT stalls
                if kk == 1 and gi >= 1:
                    emit_tailred(gi - 1, group_tails[gi - 1], 0, TGROUP // 2)
                if kk == 3 and gi >= 1:
                    emit_tailred(gi - 1, group_tails.pop(gi - 1), TGROUP // 2, TGROUP)
            emit_fold(*pending_fold)
            emit_tailred(ngroups - 1, group_tails.pop(ngroups - 1))
            nc.sync.dma_start(out[:], out_s[:])
    nc.compile()
    return nc


def _split16(v):
    h = v.astype(np.float16)
    l = (v - h.astype(np.float32)).astype(np.float16)
    return h, l


def _lhs_aug(p):
    # p: [n, 3] fp32 -> [13, n] fp16 hi/lo split; pairs with _rhs_aug rows so
    # that sum_k lhs[k,n]*rhs[k,m] = ||x||^2 + ||y||^2 - 2 x.y to ~fp32 accuracy
    p2 = np.sum(p * p, axis=-1)
    c = p.T
    ch, cl = _split16(c)
    p2h, p2l = _split16(p2)
    ones = np.ones_like(p2, dtype=np.float16)
    rows = [
        -2 * ch[0], -2 * ch[1], -2 * ch[2],  # * y_h
        -2 * ch[0], -2 * ch[1], -2 * ch[2],  # * y_l
        -2 * cl[0], -2 * cl[1], -2 * cl[2],  # * y_h
        p2h, p2l, ones, ones,
    ]
    return np.ascontiguousarray(np.stack([r.astype(np.float16) for r in rows], 0))


def _rhs_aug(p):
    p2 = np.sum(p * p, axis=-1)
    c = p.T
    ch, cl = _split16(c)
    p2h, p2l = _split16(p2)
    ones = np.ones_like(p2, dtype=np.float16)
    rows = [
        ch[0], ch[1], ch[2],
        cl[0], cl[1], cl[2],
        ch[0], ch[1], ch[2],
        ones, ones, p2h, p2l,
    ]
    return np.ascontiguousarray(np.stack([r.astype(np.float16) for r in rows], 0))


def _run(x, y, trace=False):
    if "nc" not in _NC_CACHE:
        _NC_CACHE["nc"] = build_bass()
    nc = _NC_CACHE["nc"]

    in_maps = []
    for c in range(N_CORES):
        b, h = divmod(c, 2)
        xs = x[b, h * HALF : (h + 1) * HALF]
        ys = y[b, h * HALF : (h + 1) * HALF]
        in_maps.append(
            {
                "la": _lhs_aug(xs),
                "ra0": np.ascontiguousarray(_rhs_aug(y[b])[:, : M // 2]),
                "ra1": np.ascontiguousarray(_rhs_aug(y[b])[:, M // 2 :]),
                "lb": _lhs_aug(ys),
                "rb0": np.ascontiguousarray(_rhs_aug(x[b])[:, : M // 2]),
                "rb1": np.ascontiguousarray(_rhs_aug(x[b])[:, M // 2 :]),
            }
        )
    return run_bass_kernel_spmd(nc, in_maps, list(range(N_CORES)), trace=trace)


def kernel(x, y, bidirectional):
    x = np.asarray(x, dtype=np.float32)
    y = np.asarray(y, dtype=np.float32)
    bidir = int(np.asarray(bidirectional))

    try:
        res = _run(x, y).results
    except Exception:
        # transient device failures (e.g. a previously wedged NeuronCore)
        # have been observed to clear after a short wait; retry once
        import time

        time.sleep(30)
        res = _run(x, y).results

    term1 = np.zeros(B, dtype=np.float64)
    term2 = np.zeros(B, dtype=np.float64)
    for c in range(N_CORES):
        b, h = divmod(c, 2)
        o = np.asarray(res[c]["out"])
        rma = o[:, :NT].T.reshape(-1)  # d2 row mins, x-half -> over all y
        rmb = o[:, NT:].T.reshape(-1)  # d2 row mins, y-half -> over all x
        term1[b] += np.sqrt(np.maximum(rma, 0.0)).mean() / 2.0
        term2[b] += np.sqrt(np.maximum(rmb, 0.0)).mean() / 2.0

    loss = term1.mean()
    if bidir:
        loss = loss + term2.mean()
    return np.float32(loss)


# revision 56
# speedup vs baseline: 1.0188x; 1.0188x over previous
"""Chamfer distance kernel for Trainium2 (8 NeuronCores).

Problem: x, y ~ (B=4, N=8192, 3) fp32. loss = mean_b[ mean_n min_m dist + mean_m min_n dist ].

Strategy:
  - d2[n,m] = ||x||^2 + ||y||^2 - 2 x.y as ONE augmented matmul: the fp32
    values are split hi/lo into fp16 (exact products in PSUM fp32), giving
    K=13 fp16 contraction rows -> 4x faster PE than fp32, ~1e-7 accuracy.
  - K=13 <= 32, so 4 independent matmuls are packed into the PE array via
    tile_position row-groups (base partitions 0/32/64/96) -> ~4x concurrency.
  - min over the free axis; sqrt/means on host (monotone => sqrt after min).
  - Two consumer lanes, balanced so DVE and ACT both stay busy. Every block
    (n-tile x full 8192 cols) is handled identically to avoid convoying:
      4 chunks x 1536: ScalarE Relu-cast-copies PSUM->SBUF fp16; DVE then
                       folds the 6144 cols with tensor_tensor min (fp16 2x)
      2 chunks x 1024: DVE reduce_min straight from PSUM
    Per-block partials land in a tails buffer; one batched reduce per 8
    blocks produces the final row-mins. Folds are emitted one block late so
    DVE frees PSUM promptly (keeps the ACT chunk stream gap-free).
    (tensor_tensor_reduce would be 1 instr/block but crashes this runtime)
  - 8 cores = (batch b, half h): each core does both directions for its half
    of the rows against the full opposite set => no cross-core reduction.
"""

import sys

sys.path.insert(0, "/opt/trn_rl_repo")

import numpy as np

import concourse.bacc as bacc
import concourse.mybir as mybir
from concourse import tile
from concourse.bass_utils import run_bass_kernel_spmd

N_CORES = 8
B, N, M, D = 4, 8192, 8192, 3
HALF = N // 2  # rows per core per direction
NT = HALF // 128  # 32 n-tiles per pass
import os as _os

K = 13  # augmented fp16 hi/lo contraction rows
# per-block chunk plan: ACT copies ACT_N x ACT_CHUNK cols PSUM->SBUF fp16,
# DVE reduces DIR_N x DIR_CHUNK cols directly from PSUM.
# PSUM banks: ACT_N needs 2 bufs x ACT_CHUNK/512 banks, DIR 2 x DIR_CHUNK/512.
ACT_CHUNK = int(_os.environ.get("CHAMFER_ACT_CHUNK", "1536"))
ACT_N = int(_os.environ.get("CHAMFER_ACT_N", "4"))
DIR_CHUNK = int(_os.environ.get("CHAMFER_DIR_CHUNK", "1024"))
DIR_N = int(_os.environ.get("CHAMFER_DIR_N", "2"))
assert ACT_CHUNK * ACT_N + DIR_CHUNK * DIR_N == M
STAGE_BUFS = int(_os.environ.get("CHAMFER_STAGE_BUFS", "2"))
TGROUP = int(_os.environ.get("CHAMFER_TGROUP", "8"))  # blocks per batched tail-reduce
F32 = mybir.dt.float32
F16 = mybir.dt.float16

_NC_CACHE = {}


def build_bass():
    nc = bacc.Bacc(
        "TRN2", target_bir_lowering=False, debug=False, num_devices=N_CORES
    )
    la = nc.dram_tensor("la", [K, HALF], F16, kind="ExternalInput")
    ra = nc.dram_tensor("ra", [K, M], F16, kind="ExternalInput")
    lb = nc.dram_tensor("lb", [K, HALF], F16, kind="ExternalInput")
    rb = nc.dram_tensor("rb", [K, M], F16, kind="ExternalInput")
    out = nc.dram_tensor("out", [128, 2 * NT], F32, kind="ExternalOutput")

    with tile.TileContext(nc) as tc:
        with (
            tc.tile_pool(name="inp", bufs=1) as inp,
            tc.tile_pool(name="psa", bufs=2, space="PSUM") as psa,
            tc.tile_pool(
                name="psd",
                bufs=int(_os.environ.get("CHAMFER_DIR_BUFS", "1")),
                space="PSUM",
            ) as psd,
            tc.tile_pool(name="stg", bufs=STAGE_BUFS) as stg,
            tc.tile_pool(name="foldp", bufs=int(_os.environ.get("CHAMFER_FOLD_BUFS", "2"))) as foldp,
            tc.tile_pool(name="tailp", bufs=int(_os.environ.get("CHAMFER_TAIL_BUFS", "2"))) as tailp,
            tc.tile_pool(name="res", bufs=1) as resp,
        ):
            # warm the ACT table (Relu set) while input DMAs run
            warm = inp.tile([128, 1], F32, tag="warm")
            nc.vector.memset(warm[:], 0.0)
            nc.scalar.activation(warm[:], warm[:], mybir.ActivationFunctionType.Relu)

            # inputs replicated at base partitions 0/32/64/96 for row-group
            # packed matmuls (4 concurrent MMs in the PE array)
            ls_a = inp.tile([128, HALF], F16, tag="la")
            rs_a = inp.tile([128, M], F16, tag="ra")
            ls_b = inp.tile([128, HALF], F16, tag="lb")
            rs_b = inp.tile([128, M], F16, tag="rb")
            for g in range(4):  # pass-A inputs first: block 0 starts sooner
                p0 = 32 * g
                nc.sync.dma_start(ls_a[p0 : p0 + K, :], la[:])
                nc.sync.dma_start(rs_a[p0 : p0 + K, :], ra[:])
            for g in range(4):
                p0 = 32 * g
                nc.sync.dma_start(ls_b[p0 : p0 + K, :], lb[:])
                nc.sync.dma_start(rs_b[p0 : p0 + K, :], rb[:])

            out_s = resp.tile([128, 2 * NT], F32)

            ndirect = DIR_N
            sw = ACT_N * ACT_CHUNK
            tailw = sw >> 4  # 4 fp16 TT-min fold levels
            blocks = [(p, t) for p in range(2) for t in range(NT)]
            ngroups = len(blocks) // TGROUP

            def emit_fold(stile, tails, kk):
                fold = foldp.tile([128, sw // 2], F16, tag="fold")
                cur, cw = stile, sw
                for _ in range(4):
                    half = cw // 2
                    dst = (
                        tails[:, kk, ndirect : ndirect + tailw]
                        if half == tailw
                        else fold[:, :half]
                    )
                    nc.vector.tensor_tensor(
                        dst, cur[:, :half], cur[:, half:cw], op=mybir.AluOpType.min
                    )
                    cur, cw = fold, half

            def emit_tailred(gi, tails, lo=0, hi=TGROUP):
                nc.vector.tensor_reduce(
                    out_s[:, gi * TGROUP + lo : gi * TGROUP + hi],
                    tails[:, lo:hi, :],
                    axis=mybir.AxisListType.X,
                    op=mybir.AluOpType.min,
                )

            # software pipeline: folds run one block late, tail-reduces one
            # block into the next group, so DVE frees PSUM chunks promptly
            pending_fold = None
            group_tails = {}
            for idx, (p, t) in enumerate(blocks):
                gi, kk = divmod(idx, TGROUP)
                if kk == 0:
                    group_tails[gi] = tailp.tile(
                        [128, TGROUP, ndirect + tailw], F16, tag="tails", name="tails"
                    )
                tails = group_tails[gi]
                lt, rt = (ls_a, rs_a) if p == 0 else (ls_b, rs_b)
                stile = stg.tile([128, sw], F16, tag="stage")
                gseq = 0
                acts = [("act", j, ACT_CHUNK, j * ACT_CHUNK) for j in range(ACT_N)]
                dves = [
                    ("dve", j, DIR_CHUNK, sw + j * DIR_CHUNK) for j in range(DIR_N)
                ]
                order = _os.environ.get("CHAMFER_ORDER", "head")
                if order == "tail":
                    chunks = acts + dves
                elif order == "head":
                    chunks = dves + acts
                else:  # interleave: a a d a a d
                    chunks = acts[:2] + dves[:1] + acts[2:] + dves[1:]
                for kind, j, csz, c0 in chunks:
                    if kind == "act":
                        ps = psa.tile([128, ACT_CHUNK], F32, name="psa_t", tag="psa_t")
                    else:
                        ps = psd.tile([128, DIR_CHUNK], F32, name="psd_t", tag="psd_t")
                    for s in range(csz // 512):
                        g = gseq % 4  # rotate PE row-groups
                        gseq += 1
                        p0 = 32 * g
                        cs = c0 + s * 512
                        nc.tensor.matmul(
                            ps[:, s * 512 : (s + 1) * 512],
                            lt[p0 : p0 + K, t * 128 : (t + 1) * 128],
                            rt[p0 : p0 + K, cs : cs + 512],
                            tile_position=(p0, 0),
                        )
                    if kind == "act":
                        nc.scalar.activation(
                            stile[:, j * ACT_CHUNK : (j + 1) * ACT_CHUNK],
                            ps[:],
                            mybir.ActivationFunctionType.Relu,
                        )
                    else:
                        nc.vector.tensor_reduce(
                            tails[:, kk, j : j + 1],
                            ps[:],
                            axis=mybir.AxisListType.X,
                            op=mybir.AluOpType.min,
                        )
                if pending_fold is not None:
                    emit_fold(*pending_fold)
                pending_fold = (stile, tails, kk)
                # split each group's tail reduce into two halves emitted on
                # different blocks: smaller DVE bursts -> fewer ACT stalls
                if kk == 1 and gi >= 1:
                    emit_tailred(gi - 1, group_tails[gi - 1], 0, TGROUP // 2)
                if kk == 3 and gi >= 1:
                    emit_tailred(gi - 1, group_tails.pop(gi - 1), TGROUP // 2, TGROUP)
            emit_fold(*pending_fold)
            emit_tailred(ngroups - 1, group_tails.pop(ngroups - 1))
            nc.sync.dma_start(out[:], out_s[:])
    nc.compile()
    return nc


def _split16(v):
    h = v.astype(np.float16)
    l = (v - h.astype(np.float32)).astype(np.float16)
    return h, l


def _lhs_aug(p):
    # p: [n, 3] fp32 -> [13, n] fp16 hi/lo split; pairs with _rhs_aug rows so
    # that sum_k lhs[k,n]*rhs[k,m] = ||x||^2 + ||y||^2 - 2 x.y to ~fp32 accuracy
    p2 = np.sum(p * p, axis=-1)
    c = p.T
    ch, cl = _split16(c)
    p2h, p2l = _split16(p2)
    ones = np.ones_like(p2, dtype=np.float16)
    rows = [
        -2 * ch[0], -2 * ch[1], -2 * ch[2],  # * y_h
        -2 * ch[0], -2 * ch[1], -2 * ch[2],  # * y_l
        -2 * cl[0], -2 * cl[1], -2 * cl[2],  # * y_h
        p2h, p2l, ones, ones,
    ]
    return np.ascontiguousarray(np.stack([r.astype(np.float16) for r in rows], 0))


def _rhs_aug(p):
    p2 = np.sum(p * p, axis=-1)
    c = p.T
    ch, cl = _split16(c)
    p2h, p2l = _split16(p2)
    ones = np.ones_like(p2, dtype=np.float16)
    rows = [
        ch[0], ch[1], ch[2],
        cl[0], cl[1], cl[2],
        ch[0], ch[1], ch[2],
        ones, ones, p2h, p2l,
    ]
    return np.ascontiguousarray(np.stack([r.astype(np.float16) for r in rows], 0))


def _run(x, y, trace=False):
    if "nc" not in _NC_CACHE:
        _NC_CACHE["nc"] = build_bass()
    nc = _NC_CACHE["nc"]

    in_maps = []
    for c in range(N_CORES):
        b, h = divmod(c, 2)
        xs = x[b, h * HALF : (h + 1) * HALF]
        ys = y[b, h * HALF : (h + 1) * HALF]
        in_maps.append(
            {
                "la": _lhs_aug(xs),
                "ra": _rhs_aug(y[b]),
                "lb": _lhs_aug(ys),
                "rb": _rhs_aug(x[b]),
            }
        )
    return run_bass_kernel_spmd(nc, in_maps, list(range(N_CORES)), trace=trace)


def kernel(x, y, bidirectional):
    x = np.asarray(x, dtype=np.float32)
    y = np.asarray(y, dtype=np.float32)
    bidir = int(np.asarray(bidirectional))

    try:
        res = _run(x, y).results
    except Exception:
        # transient device failures (e.g. a previously wedged NeuronCore)
        # have been observed to clear after a short wait; retry once
        import time

        time.sleep(30)
        res = _run(x, y).results

    term1 = np.zeros(B, dtype=np.float64)
    term2 = np.zeros(B, dtype=np.float64)
    for c in range(N_CORES):
        b, h = divmod(c, 2)
        o = np.asarray(res[c]["out"])
        rma = o[:, :NT].T.reshape(-1)  # d2 row mins, x-half -> over all y
        rmb = o[:, NT:].T.reshape(-1)  # d2 row mins, y-half -> over all x
        term1[b] += np.sqrt(np.maximum(rma, 0.0)).mean() / 2.0
        term2[b] += np.sqrt(np.maximum(rmb, 0.0)).mean() / 2.0

    loss = term1.mean()
    if bidir:
        loss = loss + term2.mean()
    return np.float32(loss)
